# revision 16
# baseline (speedup 1.0000x reference)
"""Trainium2 Bass kernel for nn_DiscriminationLoss (segment_reduce).

v7 (from v1's 87.5us baseline; HBM roofline ~53us/core):

  - Pixel-sharded over 8 cores: pred slice [8, 524288] f32, labels
    slice [524288] per core. Pixels are assigned per DMA group g:
    pixel = 128*goff + p*glen + f, so each (channel, group) pred read
    is one contiguous HBM run. Group sizes ramp 64/192/256 -> 512 so
    the first stationary tile is cast ~10us in, and ramp down at the
    end to shorten the exposed tail.
  - One-hot on DVE: tiny chunk 0 as one batched tensor_tensor against
    a gpsimd-memset iota tile; all other chunks are per-j
    nc.vector.tensor_scalar(is_equal, imm j) sweeps -
    InstTensorScalarPtr engages the DVE 4x_2p perf mode (0.26 ns/elem;
    tensor_tensor caps at 2x, scalar_tensor_tensor has no fast mode).
  - PE: 256 matmuls of N=512 (QB=16 blocks x 8 channels = 128-col
    stationary) accumulating into one PSUM bank, ~238ns/MM sustained.
    A warmup burst plus filler matmuls across the early oh-pipeline
    bubbles keep the HAM clock gate at 2.4 GHz for the whole stream.
  - Labels host-cast to bf16 (lossless for 0..32) ride the gpsimd
    SWDGE queue (their own DMA queue, so the tiny label slices are not
    starved behind multi-MiB pred transfers); pred groups alternate
    the SP/ACT hwdge queues so both hardware queues stream
    concurrently.
  - Counts via np.bincount host-side (int labels only; the heavy f32
    reduction stays on device).
  - bf16 everywhere (no 2^14 pre-scale needed; rel err ~1e-4 vs the
    2e-2 gate). ACT does the f32->bf16 cast + (c,t)->(tg,c,b) permute
    and the final PSUM->SBUF copies, keeping the DVE queue pure.
  - Host extracts the 16 diagonal b==b' sub-blocks of the [128,512]
    PSUM dump and runs the tiny O(K^2) pairwise tail in f64.
"""

import sys
import functools

sys.path.insert(0, "/opt/trn_rl_repo")

import numpy as np

C = 8
K = 32
NCORES = 8
H = W = 2048
PTOT = H * W
PCORE = PTOT // NCORES  # 524288
NBLK = PCORE // 128  # 4096 block columns
SIGMA_DIS = 3.0

QB = 16  # blocks per matmul group (stationary = 8 ch * 16 = 128 cols)
PGROUPS = [128, 384] + [512] * 6 + [384, 128]  # DMA groups (blocks), sum 4096
# label DMA slices: runs of groups (indices into PGROUPS); first covers
# chunks 0-1 exactly (blocks 0-1024 = g0+g1+g2)
LAB_RUNS = [(0, 3), (3, 8), (8, 10)]
CHUNKS = [64, 960, 992, 992, 960, 128]  # one-hot chunks (blocks), sum 4096
TT_CHUNKS = {0, 5}  # chunks done as one batched tensor_tensor
FILLERS = {0: 40}  # warm filler MMs after chunk ci's matmuls
WARM_MMS = 48  # PE warmup matmuls (trip the HAM clock gate to 2.4 GHz)


def build_nc():
    import concourse.bacc as bacc
    import concourse.tile as tile
    import concourse.mybir as mybir
    from contextlib import ExitStack

    assert sum(CHUNKS) == NBLK and sum(PGROUPS) == NBLK
    f32 = mybir.dt.float32
    bf16 = mybir.dt.bfloat16

    nc = bacc.Bacc(
        "TRN2", target_bir_lowering=False, debug=False, num_devices=NCORES
    )
    pred_ext = nc.dram_tensor("pred", [C, PCORE], f32, kind="ExternalInput")
    lab_ext = nc.dram_tensor("labels", [PCORE], bf16, kind="ExternalInput")
    # col 512 carries a warmup-psum dump so the warm MMs stay live
    out_ext = nc.dram_tensor("out_s", [128, 513], f32, kind="ExternalOutput")

    grp_starts = np.cumsum([0] + PGROUPS[:-1]).tolist()

    with tile.TileContext(nc) as tc, ExitStack() as ctx:
        const_pool = ctx.enter_context(tc.tile_pool(name="const", bufs=1))
        slab32_pool = ctx.enter_context(tc.tile_pool(name="slab32", bufs=3))
        slabh_pool = ctx.enter_context(tc.tile_pool(name="slabh", bufs=2))
        oh_pool = ctx.enter_context(tc.tile_pool(name="oh", bufs=2))
        ohtt_pool = ctx.enter_context(tc.tile_pool(name="ohtt", bufs=1))
        psum_pool = ctx.enter_context(tc.tile_pool(name="psum", bufs=1, space="PSUM"))
        out_pool = ctx.enter_context(tc.tile_pool(name="outp", bufs=1))

        # Labels: per-group pixel mapping; lbt col t of group g holds
        # label[128*goff + p*glen + (t-goff)]. Slices ride the ACT hwdge
        # queue (pred-even rides SP) in uniform-group runs.
        lbt = const_pool.tile([128, NBLK], bf16)

        def lab_slice(r0, r1):
            # groups r0..r1-1 may have mixed sizes: one DMA per uniform run
            i = r0
            while i < r1:
                j = i + 1
                while j < r1 and PGROUPS[j] == PGROUPS[i]:
                    j += 1
                goff = grp_starts[i]
                gend = grp_starts[j - 1] + PGROUPS[j - 1]
                gl = PGROUPS[i]
                nc.gpsimd.dma_start(
                    lbt[:, goff:gend].rearrange("p (g f) -> p g f", f=gl),
                    lab_ext[128 * goff : 128 * gend].rearrange(
                        "(g p f) -> p g f", p=128, f=gl
                    ),
                )
                i = j

        lab_slice(*LAB_RUNS[0])  # blocks 0-1024: covers chunks 0-1

        # iota for the TT chunks, built by gpsimd memsets (no DMA; gpsimd
        # is otherwise idle and DVE isn't using the SBUF port this early).
        iota_t = const_pool.tile([128, K * QB], bf16)
        for j in range(K):
            nc.gpsimd.memset(iota_t[:, j * QB : (j + 1) * QB], float(j + 1))

        # Warmup source: no DMA dependency so the PE busies immediately.
        warm_src = const_pool.tile([128, 128], bf16)
        nc.vector.memset(warm_src[:], 0.5)

        psum_t = psum_pool.tile([128, 512], f32)
        warm_ps = psum_pool.tile([128, 128], f32)
        warm_cnt = WARM_MMS + sum(FILLERS.values())
        warm_done = 0

        def warm_burst(n):
            nonlocal warm_done
            for _ in range(n):
                nc.tensor.matmul(
                    warm_ps[:],
                    warm_src[:],
                    warm_src[:],
                    start=(warm_done == 0),
                    stop=(warm_done == warm_cnt - 1),
                )
                warm_done += 1

        warm_burst(WARM_MMS)

        next_grp = 0
        cur_slabh = None
        cur_gstart = 0
        cur_glen = 0
        lab_emitted = 1

        def emit_group(gi):
            nonlocal cur_slabh, cur_gstart, cur_glen, lab_emitted
            if lab_emitted < len(LAB_RUNS) and gi >= LAB_RUNS[lab_emitted][0] - 4:
                lab_slice(*LAB_RUNS[lab_emitted])
                lab_emitted += 1
            gstart, glen = grp_starts[gi], PGROUPS[gi]
            eng = nc.sync if gi % 2 == 0 else nc.scalar  # two hwdge queues
            s32 = slab32_pool.tile([128, C * 512], f32, tag="s32")
            # src element [p, c, f] = pred[c, 128*gstart + p*glen + f]
            src = pred_ext[:, 128 * gstart : 128 * (gstart + glen)].rearrange(
                "c (p f) -> p c f", p=128
            )
            eng.dma_start(
                s32[:, : C * glen].rearrange("p (c f) -> p c f", c=C), src
            )
            slabh = slabh_pool.tile([128, 512 * C], bf16, tag="slabh")
            # permute (c, tg, b) -> (tg, c, b) during the bf16 cast so each
            # tg's stationary [128, 128] is a contiguous slice
            nc.scalar.activation(
                slabh[:, : glen * C].rearrange(
                    "p (tg c b) -> p tg c b", c=C, b=QB
                ),
                s32[:, : C * glen].rearrange(
                    "p (c tg b) -> p tg c b", c=C, b=QB
                ),
                mybir.ActivationFunctionType.Copy,
            )
            cur_slabh, cur_gstart, cur_glen = slabh, gstart, glen

        mm_idx = 0
        n_mms = NBLK // QB
        chunk_off = 0
        for ci, fcg in enumerate(CHUNKS):
            ntg = fcg // QB
            if ci in TT_CHUNKS:
                oh = ohtt_pool.tile([128, K * 128], bf16, tag="ohtt")
            else:
                oh = oh_pool.tile([128, K * 992], bf16, tag="oh")
            oh_r = oh[:, : K * fcg].rearrange(
                "p (tg j b) -> p tg j b", j=K, b=QB
            )  # [128, ntg, K, QB]
            if ci in TT_CHUNKS:
                in0 = (
                    lbt[:, chunk_off : chunk_off + fcg]
                    .rearrange("p (tg b) -> p tg b", b=QB)
                    .unsqueeze(2)
                    .broadcast_to([128, ntg, K, QB])
                )
                in1 = (
                    iota_t[:]
                    .rearrange("p (j b) -> p j b", b=QB)
                    .unsqueeze(1)
                    .broadcast_to([128, ntg, K, QB])
                )
                nc.vector.tensor_tensor(
                    oh_r, in0, in1, mybir.AluOpType.is_equal
                )
            else:
                in0 = lbt[:, chunk_off : chunk_off + fcg].rearrange(
                    "p (tg b) -> p tg b", b=QB
                )
                for j in range(1, K + 1):
                    nc.vector.tensor_scalar(
                        oh_r[:, :, j - 1, :],
                        in0,
                        float(j),
                        None,
                        mybir.AluOpType.is_equal,
                    )
            for tgc in range(ntg):
                g_abs = chunk_off // QB + tgc  # global group of 16 blocks
                blk0 = g_abs * QB
                while next_grp < len(PGROUPS) and blk0 >= cur_gstart + cur_glen:
                    emit_group(next_grp)
                    next_grp += 1
                tgl = (blk0 - cur_gstart) // QB
                nc.tensor.matmul(
                    psum_t[:],
                    cur_slabh[:, tgl * 128 : (tgl + 1) * 128],
                    oh[:, tgc * K * QB : (tgc + 1) * K * QB],
                    start=(mm_idx == 0),
                    stop=(mm_idx == n_mms - 1),
                )
                mm_idx += 1
            if ci in FILLERS:
                # keep the PE busy across the oh-pipeline fill bubble so the
                # HAM clock gate stays open
                warm_burst(FILLERS[ci])
            chunk_off += fcg

        # Final copies on ACT so the DVE queue stays pure one-hot work.
        outt = out_pool.tile([128, 513], f32)
        nc.scalar.activation(
            outt[:, :512], psum_t[:], mybir.ActivationFunctionType.Copy
        )
        nc.scalar.activation(
            outt[:, 512:513], warm_ps[:, 0:1], mybir.ActivationFunctionType.Copy
        )
        nc.sync.dma_start(out_ext[:], outt[:])
    nc.compile()
    return nc


@functools.lru_cache(maxsize=1)
def _get_program():
    return build_nc()


def make_in_maps(pred_flat, labels_flat):
    import ml_dtypes

    lab_bf16 = labels_flat.astype(ml_dtypes.bfloat16)
    in_maps = []
    for i in range(NCORES):
        sl = slice(i * PCORE, (i + 1) * PCORE)
        in_maps.append(
            {
                "pred": np.ascontiguousarray(pred_flat[:, sl]),
                "labels": np.ascontiguousarray(lab_bf16[sl]),
            }
        )
    return in_maps


def finish_host(parts_s, counts, num_kernel):
    """parts_s: per-core [128, 513] psum dumps; counts: [K] label histogram
    (np.bincount of the int labels). Tiny O(K^2) tail in f64."""
    r = np.sum([p[:, :512].astype(np.float64) for p in parts_s], axis=0)
    r4 = r.reshape(C, QB, K, QB)
    S = r4[:, np.arange(QB), :, np.arange(QB)].sum(axis=0)  # [C, K]
    N = counts.astype(np.float64)  # [K]
    A = N * np.sum(S * S, axis=0)  # [K]
    kk = int(num_kernel)
    A = A[:kk]
    pair = A[:, None] + A[None, :]
    Dm = np.maximum(SIGMA_DIS - np.sqrt(pair), 0.0)
    term = np.log(Dm * Dm + 1.0)
    L = float(np.sum(np.triu(term, k=1)))
    L *= (kk - 1) / kk
    return np.float32(L)


_last_results = None


def kernel(pred_similarities, regions_mask, kernel_labels, num_kernel, **kw):
    global _last_results
    from concourse.bass_utils import run_bass_kernel_spmd

    pred_flat = np.asarray(pred_similarities, dtype=np.float32).reshape(C, PTOT)
    labels_flat = np.asarray(kernel_labels, dtype=np.int32).reshape(PTOT)

    nc = _get_program()
    in_maps = make_in_maps(pred_flat, labels_flat)
    res = run_bass_kernel_spmd(nc, in_maps, list(range(NCORES)))
    _last_results = res
    parts_s = [res.results[i]["out_s"] for i in range(NCORES)]
    counts = np.bincount(labels_flat, minlength=K + 1)[1:].astype(np.float64)
    return finish_host(parts_s, counts, num_kernel)


# revision 17
# speedup vs baseline: 1.1583x; 1.1583x over previous
"""Trainium2 Bass kernel for nn_DiscriminationLoss (segment_reduce).

v7 (from v1's 87.5us baseline; HBM roofline ~53us/core):

  - Pixel-sharded over 8 cores: pred slice [8, 524288] f32, labels
    slice [524288] per core. Pixels are assigned per DMA group g:
    pixel = 128*goff + p*glen + f, so each (channel, group) pred read
    is one contiguous HBM run. Group sizes ramp 64/192/256 -> 512 so
    the first stationary tile is cast ~10us in, and ramp down at the
    end to shorten the exposed tail.
  - One-hot on DVE: tiny chunk 0 as one batched tensor_tensor against
    a gpsimd-memset iota tile; all other chunks are per-j
    nc.vector.tensor_scalar(is_equal, imm j) sweeps -
    InstTensorScalarPtr engages the DVE 4x_2p perf mode (0.26 ns/elem;
    tensor_tensor caps at 2x, scalar_tensor_tensor has no fast mode).
  - PE: 256 matmuls of N=512 (QB=16 blocks x 8 channels = 128-col
    stationary) accumulating into one PSUM bank, ~238ns/MM sustained.
    A warmup burst plus filler matmuls across the early oh-pipeline
    bubbles keep the HAM clock gate at 2.4 GHz for the whole stream.
  - Labels host-cast to bf16 (lossless for 0..32) ride the gpsimd
    SWDGE queue (their own DMA queue, so the tiny label slices are not
    starved behind multi-MiB pred transfers); pred groups alternate
    the SP/ACT hwdge queues so both hardware queues stream
    concurrently.
  - Counts via np.bincount host-side (int labels only; the heavy f32
    reduction stays on device).
  - bf16 everywhere (no 2^14 pre-scale needed; rel err ~1e-4 vs the
    2e-2 gate). ACT does the f32->bf16 cast + (c,t)->(tg,c,b) permute
    and the final PSUM->SBUF copies, keeping the DVE queue pure.
  - Host extracts the 16 diagonal b==b' sub-blocks of the [128,512]
    PSUM dump and runs the tiny O(K^2) pairwise tail in f64.
"""

import sys
import functools

sys.path.insert(0, "/opt/trn_rl_repo")

import numpy as np

C = 8
K = 32
NCORES = 8
H = W = 2048
PTOT = H * W
PCORE = PTOT // NCORES  # 524288
NBLK = PCORE // 128  # 4096 block columns
SIGMA_DIS = 3.0

QB = 16  # blocks per matmul group (stationary = 8 ch * 16 = 128 cols)
PGROUPS = [128, 384] + [512] * 6 + [384, 128]  # DMA groups (blocks), sum 4096
# label DMA slices: runs of groups (indices into PGROUPS); first covers
# chunks 0-1 exactly (blocks 0-1024 = g0+g1+g2)
LAB_RUNS = [(0, 3), (3, 8), (8, 10)]
CHUNKS = [64, 1024, 1024, 1024, 768, 192]  # one-hot chunks (blocks), sum 4096
TT_CHUNKS = {0, 5}  # chunks done as one batched tensor_tensor
FILLERS = {}  # warm filler MMs after chunk ci's matmuls (off)
WARM_MMS = 48  # PE warmup matmuls (trip the HAM clock gate to 2.4 GHz)


def build_nc():
    import concourse.bacc as bacc
    import concourse.tile as tile
    import concourse.mybir as mybir
    from contextlib import ExitStack

    assert sum(CHUNKS) == NBLK and sum(PGROUPS) == NBLK
    f32 = mybir.dt.float32
    bf16 = mybir.dt.bfloat16

    nc = bacc.Bacc(
        "TRN2", target_bir_lowering=False, debug=False, num_devices=NCORES
    )
    pred_ext = nc.dram_tensor("pred", [C, PCORE], f32, kind="ExternalInput")
    lab_ext = nc.dram_tensor("labels", [PCORE], bf16, kind="ExternalInput")
    # col 512 carries a warmup-psum dump so the warm MMs stay live
    out_ext = nc.dram_tensor("out_s", [128, 513], f32, kind="ExternalOutput")

    grp_starts = np.cumsum([0] + PGROUPS[:-1]).tolist()

    with tile.TileContext(nc) as tc, ExitStack() as ctx:
        const_pool = ctx.enter_context(tc.tile_pool(name="const", bufs=1))
        slab32_pool = ctx.enter_context(tc.tile_pool(name="slab32", bufs=2))
        slabh_pool = ctx.enter_context(tc.tile_pool(name="slabh", bufs=3))
        oh_pool = ctx.enter_context(tc.tile_pool(name="oh", bufs=2))
        psum_pool = ctx.enter_context(tc.tile_pool(name="psum", bufs=1, space="PSUM"))
        out_pool = ctx.enter_context(tc.tile_pool(name="outp", bufs=1))

        # Labels: per-group pixel mapping; lbt col t of group g holds
        # label[128*goff + p*glen + (t-goff)]. Slices ride the ACT hwdge
        # queue (pred-even rides SP) in uniform-group runs.
        lbt = const_pool.tile([128, NBLK], bf16)

        def lab_slice(r0, r1):
            # groups r0..r1-1 may have mixed sizes: one DMA per uniform run
            i = r0
            while i < r1:
                j = i + 1
                while j < r1 and PGROUPS[j] == PGROUPS[i]:
                    j += 1
                goff = grp_starts[i]
                gend = grp_starts[j - 1] + PGROUPS[j - 1]
                gl = PGROUPS[i]
                nc.sync.dma_start(
                    lbt[:, goff:gend].rearrange("p (g f) -> p g f", f=gl),
                    lab_ext[128 * goff : 128 * gend].rearrange(
                        "(g p f) -> p g f", p=128, f=gl
                    ),
                )
                i = j

        lab_slice(*LAB_RUNS[0])  # blocks 0-1024: covers chunks 0-1

        # iota for the TT chunks, built by gpsimd memsets (no DMA; gpsimd
        # is otherwise idle and DVE isn't using the SBUF port this early).
        iota_t = const_pool.tile([128, K * QB], bf16)
        for j in range(K):
            nc.gpsimd.memset(iota_t[:, j * QB : (j + 1) * QB], float(j + 1))

        # Warmup source: no DMA dependency so the PE busies immediately.
        warm_src = const_pool.tile([128, 128], bf16)
        nc.vector.memset(warm_src[:], 0.5)

        psum_t = psum_pool.tile([128, 512], f32)
        warm_ps = psum_pool.tile([128, 128], f32)
        warm_cnt = WARM_MMS + sum(FILLERS.values())
        warm_done = 0

        def warm_burst(n):
            nonlocal warm_done
            for _ in range(n):
                nc.tensor.matmul(
                    warm_ps[:],
                    warm_src[:],
                    warm_src[:],
                    start=(warm_done == 0),
                    stop=(warm_done == warm_cnt - 1),
                )
                warm_done += 1

        warm_burst(WARM_MMS)

        next_grp = 0
        cur_slabh = None
        cur_gstart = 0
        cur_glen = 0
        lab_emitted = 1

        def emit_group(gi):
            nonlocal cur_slabh, cur_gstart, cur_glen, lab_emitted
            if lab_emitted < len(LAB_RUNS) and gi >= LAB_RUNS[lab_emitted][0] - 4:
                lab_slice(*LAB_RUNS[lab_emitted])
                lab_emitted += 1
            gstart, glen = grp_starts[gi], PGROUPS[gi]
            eng = nc.sync if gi % 2 == 0 else nc.scalar  # two hwdge queues
            s32 = slab32_pool.tile([128, C * 512], f32, tag="s32")
            # src element [p, c, f] = pred[c, 128*gstart + p*glen + f]
            src = pred_ext[:, 128 * gstart : 128 * (gstart + glen)].rearrange(
                "c (p f) -> p c f", p=128
            )
            eng.dma_start(
                s32[:, : C * glen].rearrange("p (c f) -> p c f", c=C), src
            )
            slabh = slabh_pool.tile([128, 512 * C], bf16, tag="slabh")
            # permute (c, tg, b) -> (tg, c, b) during the bf16 cast so each
            # tg's stationary [128, 128] is a contiguous slice
            nc.scalar.activation(
                slabh[:, : glen * C].rearrange(
                    "p (tg c b) -> p tg c b", c=C, b=QB
                ),
                s32[:, : C * glen].rearrange(
                    "p (c tg b) -> p tg c b", c=C, b=QB
                ),
                mybir.ActivationFunctionType.Copy,
            )
            cur_slabh, cur_gstart, cur_glen = slabh, gstart, glen

        mm_idx = 0
        n_mms = NBLK // QB
        chunk_off = 0
        for ci, fcg in enumerate(CHUNKS):
            ntg = fcg // QB
            oh = oh_pool.tile([128, K * 1024], bf16, tag="oh")
            oh_r = oh[:, : K * fcg].rearrange(
                "p (tg j b) -> p tg j b", j=K, b=QB
            )  # [128, ntg, K, QB]
            if ci in TT_CHUNKS:
                in0 = (
                    lbt[:, chunk_off : chunk_off + fcg]
                    .rearrange("p (tg b) -> p tg b", b=QB)
                    .unsqueeze(2)
                    .broadcast_to([128, ntg, K, QB])
                )
                in1 = (
                    iota_t[:]
                    .rearrange("p (j b) -> p j b", b=QB)
                    .unsqueeze(1)
                    .broadcast_to([128, ntg, K, QB])
                )
                nc.vector.tensor_tensor(
                    oh_r, in0, in1, mybir.AluOpType.is_equal
                )
            else:
                in0 = lbt[:, chunk_off : chunk_off + fcg].rearrange(
                    "p (tg b) -> p tg b", b=QB
                )
                for j in range(1, K + 1):
                    nc.vector.tensor_scalar(
                        oh_r[:, :, j - 1, :],
                        in0,
                        float(j),
                        None,
                        mybir.AluOpType.is_equal,
                    )
            for tgc in range(ntg):
                g_abs = chunk_off // QB + tgc  # global group of 16 blocks
                blk0 = g_abs * QB
                while next_grp < len(PGROUPS) and blk0 >= cur_gstart + cur_glen:
                    emit_group(next_grp)
                    next_grp += 1
                tgl = (blk0 - cur_gstart) // QB
                nc.tensor.matmul(
                    psum_t[:],
                    cur_slabh[:, tgl * 128 : (tgl + 1) * 128],
                    oh[:, tgc * K * QB : (tgc + 1) * K * QB],
                    start=(mm_idx == 0),
                    stop=(mm_idx == n_mms - 1),
                )
                mm_idx += 1
            if ci in FILLERS:
                # keep the PE busy across the oh-pipeline fill bubble so the
                # HAM clock gate stays open
                warm_burst(FILLERS[ci])
            chunk_off += fcg

        # Final copies on ACT so the DVE queue stays pure one-hot work.
        outt = out_pool.tile([128, 513], f32)
        nc.scalar.activation(
            outt[:, :512], psum_t[:], mybir.ActivationFunctionType.Copy
        )
        nc.scalar.activation(
            outt[:, 512:513], warm_ps[:, 0:1], mybir.ActivationFunctionType.Copy
        )
        nc.sync.dma_start(out_ext[:], outt[:])
    nc.compile()
    return nc


@functools.lru_cache(maxsize=1)
def _get_program():
    return build_nc()


def make_in_maps(pred_flat, labels_flat):
    import ml_dtypes

    lab_bf16 = labels_flat.astype(ml_dtypes.bfloat16)
    in_maps = []
    for i in range(NCORES):
        sl = slice(i * PCORE, (i + 1) * PCORE)
        in_maps.append(
            {
                "pred": np.ascontiguousarray(pred_flat[:, sl]),
                "labels": np.ascontiguousarray(lab_bf16[sl]),
            }
        )
    return in_maps


def finish_host(parts_s, counts, num_kernel):
    """parts_s: per-core [128, 513] psum dumps; counts: [K] label histogram
    (np.bincount of the int labels). Tiny O(K^2) tail in f64."""
    r = np.sum([p[:, :512].astype(np.float64) for p in parts_s], axis=0)
    r4 = r.reshape(C, QB, K, QB)
    S = r4[:, np.arange(QB), :, np.arange(QB)].sum(axis=0)  # [C, K]
    N = counts.astype(np.float64)  # [K]
    A = N * np.sum(S * S, axis=0)  # [K]
    kk = int(num_kernel)
    A = A[:kk]
    pair = A[:, None] + A[None, :]
    Dm = np.maximum(SIGMA_DIS - np.sqrt(pair), 0.0)
    term = np.log(Dm * Dm + 1.0)
    L = float(np.sum(np.triu(term, k=1)))
    L *= (kk - 1) / kk
    return np.float32(L)


_last_results = None


def kernel(pred_similarities, regions_mask, kernel_labels, num_kernel, **kw):
    global _last_results
    from concourse.bass_utils import run_bass_kernel_spmd

    pred_flat = np.asarray(pred_similarities, dtype=np.float32).reshape(C, PTOT)
    labels_flat = np.asarray(kernel_labels, dtype=np.int32).reshape(PTOT)

    nc = _get_program()
    in_maps = make_in_maps(pred_flat, labels_flat)
    res = run_bass_kernel_spmd(nc, in_maps, list(range(NCORES)))
    _last_results = res
    parts_s = [res.results[i]["out_s"] for i in range(NCORES)]
    counts = np.bincount(labels_flat, minlength=K + 1)[1:].astype(np.float64)
    return finish_host(parts_s, counts, num_kernel)


# revision 18
# speedup vs baseline: 1.1716x; 1.0115x over previous
"""Trainium2 Bass kernel for nn_DiscriminationLoss (segment_reduce).

v7 (from v1's 87.5us baseline; HBM roofline ~53us/core):

  - Pixel-sharded over 8 cores: pred slice [8, 524288] f32, labels
    slice [524288] per core. Pixels are assigned per DMA group g:
    pixel = 128*goff + p*glen + f, so each (channel, group) pred read
    is one contiguous HBM run. Group sizes ramp 64/192/256 -> 512 so
    the first stationary tile is cast ~10us in, and ramp down at the
    end to shorten the exposed tail.
  - One-hot on DVE: tiny chunk 0 as one batched tensor_tensor against
    a gpsimd-memset iota tile; all other chunks are per-j
    nc.vector.tensor_scalar(is_equal, imm j) sweeps -
    InstTensorScalarPtr engages the DVE 4x_2p perf mode (0.26 ns/elem;
    tensor_tensor caps at 2x, scalar_tensor_tensor has no fast mode).
  - PE: 256 matmuls of N=512 (QB=16 blocks x 8 channels = 128-col
    stationary) accumulating into one PSUM bank, ~238ns/MM sustained.
    A warmup burst plus filler matmuls across the early oh-pipeline
    bubbles keep the HAM clock gate at 2.4 GHz for the whole stream.
  - Labels host-cast to bf16 (lossless for 0..32) ride the gpsimd
    SWDGE queue (their own DMA queue, so the tiny label slices are not
    starved behind multi-MiB pred transfers); pred groups alternate
    the SP/ACT hwdge queues so both hardware queues stream
    concurrently.
  - Counts via np.bincount host-side (int labels only; the heavy f32
    reduction stays on device).
  - bf16 everywhere (no 2^14 pre-scale needed; rel err ~1e-4 vs the
    2e-2 gate). ACT does the f32->bf16 cast + (c,t)->(tg,c,b) permute
    and the final PSUM->SBUF copies, keeping the DVE queue pure.
  - Host extracts the 16 diagonal b==b' sub-blocks of the [128,512]
    PSUM dump and runs the tiny O(K^2) pairwise tail in f64.
"""

import sys
import functools

sys.path.insert(0, "/opt/trn_rl_repo")

import numpy as np

C = 8
K = 32
NCORES = 8
H = W = 2048
PTOT = H * W
PCORE = PTOT // NCORES  # 524288
NBLK = PCORE // 128  # 4096 block columns
SIGMA_DIS = 3.0

QB = 16  # blocks per matmul group (stationary = 8 ch * 16 = 128 cols)
PGROUPS = [128, 384] + [512] * 6 + [384, 128]  # DMA groups (blocks), sum 4096
# label DMA slices: runs of groups (indices into PGROUPS)
LAB_RUNS = [(0, 1), (1, 2), (2, 8), (8, 9), (9, 10)]
CHUNKS = [64, 1024, 1024, 1024, 768, 192]  # one-hot chunks (blocks), sum 4096
TT_CHUNKS = {0, 5}  # chunks done as one batched tensor_tensor
FILLERS = {}  # warm filler MMs after chunk ci's matmuls (off)
WARM_MMS = 48  # PE warmup matmuls (trip the HAM clock gate to 2.4 GHz)


def build_nc():
    import concourse.bacc as bacc
    import concourse.tile as tile
    import concourse.mybir as mybir
    from contextlib import ExitStack

    assert sum(CHUNKS) == NBLK and sum(PGROUPS) == NBLK
    f32 = mybir.dt.float32
    bf16 = mybir.dt.bfloat16

    nc = bacc.Bacc(
        "TRN2", target_bir_lowering=False, debug=False, num_devices=NCORES
    )
    pred_ext = nc.dram_tensor("pred", [C, PCORE], f32, kind="ExternalInput")
    lab_ext = nc.dram_tensor("labels", [PCORE], bf16, kind="ExternalInput")
    # col 512 carries a warmup-psum dump so the warm MMs stay live
    out_ext = nc.dram_tensor("out_s", [128, 513], f32, kind="ExternalOutput")

    grp_starts = np.cumsum([0] + PGROUPS[:-1]).tolist()

    with tile.TileContext(nc) as tc, ExitStack() as ctx:
        const_pool = ctx.enter_context(tc.tile_pool(name="const", bufs=1))
        slab32_pool = ctx.enter_context(tc.tile_pool(name="slab32", bufs=2))
        slabh_pool = ctx.enter_context(tc.tile_pool(name="slabh", bufs=3))
        oh_pool = ctx.enter_context(tc.tile_pool(name="oh", bufs=2))
        psum_pool = ctx.enter_context(tc.tile_pool(name="psum", bufs=1, space="PSUM"))
        out_pool = ctx.enter_context(tc.tile_pool(name="outp", bufs=1))

        # Labels: per-group pixel mapping; lbt col t of group g holds
        # label[128*goff + p*glen + (t-goff)]. Slices ride the ACT hwdge
        # queue (pred-even rides SP) in uniform-group runs.
        lbt = const_pool.tile([128, NBLK], bf16)

        def lab_slice(r0, r1):
            # groups r0..r1-1 may have mixed sizes: one DMA per uniform run
            i = r0
            while i < r1:
                j = i + 1
                while j < r1 and PGROUPS[j] == PGROUPS[i]:
                    j += 1
                goff = grp_starts[i]
                gend = grp_starts[j - 1] + PGROUPS[j - 1]
                gl = PGROUPS[i]
                nc.sync.dma_start(
                    lbt[:, goff:gend].rearrange("p (g f) -> p g f", f=gl),
                    lab_ext[128 * goff : 128 * gend].rearrange(
                        "(g p f) -> p g f", p=128, f=gl
                    ),
                )
                i = j

        for r in LAB_RUNS:
            lab_slice(*r)

        # iota for the TT chunks, built by gpsimd memsets (no DMA; gpsimd
        # is otherwise idle and DVE isn't using the SBUF port this early).
        iota_t = const_pool.tile([128, K * QB], bf16)
        for j in range(K):
            nc.gpsimd.memset(iota_t[:, j * QB : (j + 1) * QB], float(j + 1))

        # Warmup source: no DMA dependency so the PE busies immediately.
        warm_src = const_pool.tile([128, 128], bf16)
        nc.vector.memset(warm_src[:], 0.5)

        psum_t = psum_pool.tile([128, 512], f32)
        warm_ps = psum_pool.tile([128, 128], f32)
        warm_cnt = WARM_MMS + sum(FILLERS.values())
        warm_done = 0

        def warm_burst(n):
            nonlocal warm_done
            for _ in range(n):
                nc.tensor.matmul(
                    warm_ps[:],
                    warm_src[:],
                    warm_src[:],
                    start=(warm_done == 0),
                    stop=(warm_done == warm_cnt - 1),
                )
                warm_done += 1

        warm_burst(WARM_MMS)

        next_grp = 0
        cur_slabh = None
        cur_gstart = 0
        cur_glen = 0
        def emit_group(gi):
            nonlocal cur_slabh, cur_gstart, cur_glen
            gstart, glen = grp_starts[gi], PGROUPS[gi]
            eng = nc.sync if gi % 2 == 0 else nc.scalar  # two hwdge queues
            s32 = slab32_pool.tile([128, C * 512], f32, tag="s32")
            # src element [p, c, f] = pred[c, 128*gstart + p*glen + f]
            src = pred_ext[:, 128 * gstart : 128 * (gstart + glen)].rearrange(
                "c (p f) -> p c f", p=128
            )
            eng.dma_start(
                s32[:, : C * glen].rearrange("p (c f) -> p c f", c=C), src
            )
            slabh = slabh_pool.tile([128, 512 * C], bf16, tag="slabh")
            # permute (c, tg, b) -> (tg, c, b) during the bf16 cast so each
            # tg's stationary [128, 128] is a contiguous slice
            nc.scalar.activation(
                slabh[:, : glen * C].rearrange(
                    "p (tg c b) -> p tg c b", c=C, b=QB
                ),
                s32[:, : C * glen].rearrange(
                    "p (c tg b) -> p tg c b", c=C, b=QB
                ),
                mybir.ActivationFunctionType.Copy,
            )
            cur_slabh, cur_gstart, cur_glen = slabh, gstart, glen

        mm_idx = 0
        n_mms = NBLK // QB
        chunk_off = 0
        for ci, fcg in enumerate(CHUNKS):
            ntg = fcg // QB
            oh = oh_pool.tile([128, K * 1024], bf16, tag="oh")
            oh_r = oh[:, : K * fcg].rearrange(
                "p (tg j b) -> p tg j b", j=K, b=QB
            )  # [128, ntg, K, QB]
            if ci in TT_CHUNKS:
                in0 = (
                    lbt[:, chunk_off : chunk_off + fcg]
                    .rearrange("p (tg b) -> p tg b", b=QB)
                    .unsqueeze(2)
                    .broadcast_to([128, ntg, K, QB])
                )
                in1 = (
                    iota_t[:]
                    .rearrange("p (j b) -> p j b", b=QB)
                    .unsqueeze(1)
                    .broadcast_to([128, ntg, K, QB])
                )
                nc.vector.tensor_tensor(
                    oh_r, in0, in1, mybir.AluOpType.is_equal
                )
            else:
                in0 = lbt[:, chunk_off : chunk_off + fcg].rearrange(
                    "p (tg b) -> p tg b", b=QB
                )
                for j in range(1, K + 1):
                    nc.vector.tensor_scalar(
                        oh_r[:, :, j - 1, :],
                        in0,
                        float(j),
                        None,
                        mybir.AluOpType.is_equal,
                    )
            for tgc in range(ntg):
                g_abs = chunk_off // QB + tgc  # global group of 16 blocks
                blk0 = g_abs * QB
                while next_grp < len(PGROUPS) and blk0 >= cur_gstart + cur_glen:
                    emit_group(next_grp)
                    next_grp += 1
                tgl = (blk0 - cur_gstart) // QB
                nc.tensor.matmul(
                    psum_t[:],
                    cur_slabh[:, tgl * 128 : (tgl + 1) * 128],
                    oh[:, tgc * K * QB : (tgc + 1) * K * QB],
                    start=(mm_idx == 0),
                    stop=(mm_idx == n_mms - 1),
                )
                mm_idx += 1
            if ci in FILLERS:
                # keep the PE busy across the oh-pipeline fill bubble so the
                # HAM clock gate stays open
                warm_burst(FILLERS[ci])
            chunk_off += fcg

        # Final copies on ACT so the DVE queue stays pure one-hot work.
        outt = out_pool.tile([128, 513], f32)
        nc.scalar.activation(
            outt[:, :512], psum_t[:], mybir.ActivationFunctionType.Copy
        )
        nc.scalar.activation(
            outt[:, 512:513], warm_ps[:, 0:1], mybir.ActivationFunctionType.Copy
        )
        nc.sync.dma_start(out_ext[:], outt[:])
    nc.compile()
    return nc


@functools.lru_cache(maxsize=1)
def _get_program():
    return build_nc()


def make_in_maps(pred_flat, labels_flat):
    import ml_dtypes

    lab_bf16 = labels_flat.astype(ml_dtypes.bfloat16)
    in_maps = []
    for i in range(NCORES):
        sl = slice(i * PCORE, (i + 1) * PCORE)
        in_maps.append(
            {
                "pred": np.ascontiguousarray(pred_flat[:, sl]),
                "labels": np.ascontiguousarray(lab_bf16[sl]),
            }
        )
    return in_maps


def finish_host(parts_s, counts, num_kernel):
    """parts_s: per-core [128, 513] psum dumps; counts: [K] label histogram
    (np.bincount of the int labels). Tiny O(K^2) tail in f64."""
    r = np.sum([p[:, :512].astype(np.float64) for p in parts_s], axis=0)
    r4 = r.reshape(C, QB, K, QB)
    S = r4[:, np.arange(QB), :, np.arange(QB)].sum(axis=0)  # [C, K]
    N = counts.astype(np.float64)  # [K]
    A = N * np.sum(S * S, axis=0)  # [K]
    kk = int(num_kernel)
    A = A[:kk]
    pair = A[:, None] + A[None, :]
    Dm = np.maximum(SIGMA_DIS - np.sqrt(pair), 0.0)
    term = np.log(Dm * Dm + 1.0)
    L = float(np.sum(np.triu(term, k=1)))
    L *= (kk - 1) / kk
    return np.float32(L)


_last_results = None


def kernel(pred_similarities, regions_mask, kernel_labels, num_kernel, **kw):
    global _last_results
    from concourse.bass_utils import run_bass_kernel_spmd

    pred_flat = np.asarray(pred_similarities, dtype=np.float32).reshape(C, PTOT)
    labels_flat = np.asarray(kernel_labels, dtype=np.int32).reshape(PTOT)

    nc = _get_program()
    in_maps = make_in_maps(pred_flat, labels_flat)
    res = run_bass_kernel_spmd(nc, in_maps, list(range(NCORES)))
    _last_results = res
    parts_s = [res.results[i]["out_s"] for i in range(NCORES)]
    counts = np.bincount(labels_flat, minlength=K + 1)[1:].astype(np.float64)
    return finish_host(parts_s, counts, num_kernel)
